# revision 4
# baseline (speedup 1.0000x reference)
"""Two-layer GAT (8-head 2->128, then 1-head 128->4 + log_softmax) on 8 TRN2 cores.

Strategy: destination-node sharding with degree-sorted 128-row ELL tiles
(as the previous version), but per-edge source features are fetched with
bulk `dma_gather` (InstDMAGatherAnt, mlp Q7 library) instead of per-slot
indirect DMAs: one call gathers ~8192 edges (~64 slot-columns) at ~2ns/idx
of GPSIMD time vs ~1us per 128-edge indirect DMA.

dma_gather constraints engineered around:
  - int16 indices -> the flat [N,16]-f32 node table is viewed as [N/4, 64]
    (256B rows, 4 nodes each): idx = node//4 <= 25088. The wanted 64B
    sub-entry is extracted on DVE with 4 host-precomputed one-hot masks.
  - elem_size must be a 256B multiple -> 256B/edge HBM traffic.
  - <=64 descriptors/packet -> single_packet=False.
  - SWDGE ring capacity -> calls of <=8192 idx on 4 rotating queues,
    dynamic_dma_scratch_size=32768.

Tables (f32): layer 1: Z1[n] = [a_src1(8) | x0 x1 | 0*6]  (a_src1 = x @ As)
              layer 2: T2[q] = [h3(4) | a_src2 | junk*11] (q = core-major pos)
Dummy entry at node id N_pad (row 25088, sub 0): a_src = -1e30 (finite, so
0*mask stays 0, exp -> 0); its other sub-entries are zeros. Junk table
columns are never extracted (mask passes read only the used columns of each
sub-entry). Segment softmax max-subtraction skipped (value ranges small).
Layer-1 aggregation uses rank-2 structure of h1 = x @ W1: only sums of
alpha*x (2 cols) are reduced per dst, then expanded through W1 with one PE
matmul per block of <=4 tiles. An AllGather shares the T2 table between
layers.
"""

import os
import numpy as np
from contextlib import ExitStack

import concourse.bass as bass
import concourse.bacc as bacc
import concourse.tile as tile
from concourse import mybir
from concourse.bass import AP
from concourse.bass_utils import run_bass_kernel_spmd

P = 128
NCORE = 8
NEG = 0.2
EPS = 1e-16
NEGBIG = -1.0e30
F32 = mybir.dt.float32
I16 = mybir.dt.int16

# consts column map
AS0X, AS1X, AD0, AD1 = 0, 16, 32, 40
W1BLK, W2EXT, B2, B1, IDENT = 48, 176, 182, 186, 187
CW = 320

BLK_COLS = 64          # max ELL slot-columns per dma_gather call (8192 idx)
BLK_TILES = 4          # max tiles per block (PSUM: 4*128 <= 512 f32)
TQW = BLK_COLS * 10    # f32 stride between extraction temps


def _v(t_ap: AP, off: int, dims) -> AP:
    """View with t_ap's partition dim and custom free dims [[step,count],...]."""
    return AP(t_ap.tensor, t_ap.offset + off, [list(t_ap.ap[0])] + [list(d) for d in dims])


def _dv(handle, off: int, dims) -> AP:
    """DRAM view with custom dims."""
    base = handle[:]
    return AP(base.tensor, off, [list(d) for d in dims])


def _plan(src: np.ndarray, dst: np.ndarray, N: int):
    """Host-side index-only preprocessing: degree sort, ELL tiling, gather
    index/mask arrays, block partition."""
    E = src.shape[0]
    deg = np.bincount(dst, minlength=N).astype(np.int64)
    T = int(np.ceil(N / (P * NCORE)))          # local tiles per core
    NT = T * NCORE
    N_pad = NT * P
    assert N_pad % 4 == 0
    order = np.concatenate([np.argsort(-deg, kind="stable"), np.arange(N, N_pad)])
    deg_pad = np.concatenate([deg, np.zeros(N_pad - N, np.int64)])
    odeg = deg_pad[order]
    tile_max = odeg.reshape(NT, P).max(axis=1)           # [NT] global tiles
    D_i = np.maximum(tile_max.reshape(T, NCORE).max(axis=1), 1)  # [T]

    # one group per tile: (i0, D, off)
    groups = []
    off = 0
    for t in range(T):
        groups.append((t, int(D_i[t]), off))
        off += int(D_i[t])
    S = off

    colbase = np.array([g[2] for g in groups], np.int64)

    # blocks: consecutive groups, total width <= BLK_COLS, <= BLK_TILES tiles
    blocks = []  # (g0, ng, W, coff)
    g0 = 0
    while g0 < T:
        w = 0
        ng = 0
        while (g0 + ng < T and ng < BLK_TILES
               and w + groups[g0 + ng][1] <= BLK_COLS):
            w += groups[g0 + ng][1]
            ng += 1
        assert ng > 0
        blocks.append((g0, ng, w, groups[g0][2]))
        g0 += ng

    inv_order = np.empty(N_pad, np.int64)
    inv_order[order] = np.arange(N_pad)

    # pos2: row of node n in the (chunked) allgathered T2 table (core-major)
    q = np.arange(N_pad)
    g = q // P
    pos_of_q = (g % NCORE) * (T * P) + (g // NCORE) * P + (q % P)
    pos2 = np.empty(N_pad, np.int64)
    pos2[order[q]] = pos_of_q

    # edges sorted by dst
    eorder = np.argsort(dst, kind="stable")
    dsts = dst[eorder]
    srcs = src[eorder]
    csr = np.zeros(N + 1, np.int64)
    csr[1:] = np.cumsum(deg)
    j = np.arange(E) - csr[dsts]              # rank within dst segment
    qe = inv_order[dsts]
    ge = qe // P
    de = qe % P
    ce = ge % NCORE
    ie = ge // NCORE
    cole = colbase[ie] + j

    # per-slot source node id (layer1) / table position (layer2); dummy N_pad
    sid1 = np.full((NCORE, P, S), N_pad, np.int64)
    sid2 = np.full((NCORE, P, S), N_pad, np.int64)
    sid1[ce, de, cole] = srcs
    sid2[ce, de, cole] = pos2[srcs]

    def pack(sid):
        """-> idx blob [NCORE, P, S*8] i16 (wrapped-16, replicated across the
        8 partition groups, slot-column-major g = w*128 + d) and mask blob
        [NCORE, P, S*4] f32 (one-hot of sub-entry)."""
        row = (sid // 4).astype(np.int16)          # [C, P, S]
        sub = (sid % 4).astype(np.int64)
        idxb = np.zeros((NCORE, P, S * 8), np.int16)
        for c in range(NCORE):
            flat = row[c].T.reshape(S * P)          # g = w*128 + d major
            wrap = flat.reshape(S * 8, 16).T        # [16, S*8]
            for grp in range(8):
                idxb[c, grp * 16:(grp + 1) * 16, :] = wrap
        mskb = np.zeros((NCORE, P, S, 4), np.float32)
        cc, dd, ww = np.meshgrid(np.arange(NCORE), np.arange(P), np.arange(S),
                                 indexing="ij")
        mskb[cc, dd, ww, sub] = 1.0
        return idxb, mskb.reshape(NCORE, P, S * 4)

    idx1, msk1 = pack(sid1)
    idx2, msk2 = pack(sid2)

    # dst node ids per (core, partition, local tile)
    og = order.reshape(NT, P)                  # [g, d]
    dstid = np.empty((NCORE, P, T), np.int64)
    for c in range(NCORE):
        dstid[c] = og[c::NCORE].transpose(1, 0)  # [P, T]

    return dict(E=E, T=T, N_pad=N_pad, S=S, groups=groups, blocks=blocks,
                order=order, pos_of_q=pos_of_q, dstid=dstid,
                idx1=idx1, msk1=msk1, idx2=idx2, msk2=msk2)


def _consts(W1, att_src1, att_dst1, b1, W2, att_src2, att_dst2, b2):
    W1r = W1.reshape(2, 8, 16)
    As = np.einsum("khc,hc->kh", W1r, att_src1)    # [2, 8]
    Ad = np.einsum("khc,hc->kh", W1r, att_dst1)
    c = np.zeros((P, CW), np.float32)
    c[:, AS0X:AS0X + 8] = As[0]
    c[:, AS0X + 8] = 1.0
    c[:, AS1X:AS1X + 8] = As[1]
    c[:, AS1X + 9] = 1.0
    c[:, AD0:AD0 + 8] = Ad[0]
    c[:, AD1:AD1 + 8] = Ad[1]
    w1blk = np.zeros((16, 128), np.float32)
    for k in range(2):
        for h in range(8):
            w1blk[k * 8 + h, h * 16:(h + 1) * 16] = W1r[k, h]
    c[:16, W1BLK:W1BLK + 128] = w1blk
    c[:, W2EXT:W2EXT + 4] = W2
    c[:, W2EXT + 4] = W2 @ att_src2[0]
    c[:, W2EXT + 5] = W2 @ att_dst2[0]
    c[:, B2:B2 + 4] = b2
    c[:, B1] = b1
    c[:, IDENT:IDENT + 128] = np.eye(P, dtype=np.float32)
    # dummy table rows (one 256B row = 4 sub-entries; only sub 0 selected)
    dum1 = np.zeros((1, 64), np.float32)
    dum1[0, 0:8] = NEGBIG       # Z1 dummy: a_src = -BIG, x = 0
    dum2 = np.zeros((1, 64), np.float32)
    dum2[0, 4] = NEGBIG         # T2 dummy: h3 = 0, a_src2 = -BIG
    return c, dum1, dum2


def _build(T, S, groups, blocks, N_pad, use_prelu=True):
    R1 = N_pad // 4 + 1         # gather-table rows incl. dummy
    nc = bacc.Bacc("TRN2", target_bir_lowering=False,
                   num_swdge_queues=4, dynamic_dma_scratch_size=32768)
    xin = nc.declare_dram_parameter("xpad", [N_pad, 2], F32, isOutput=False)
    idx1in = nc.declare_dram_parameter("idx1", [P, S * 8], I16, isOutput=False)
    msk1in = nc.declare_dram_parameter("msk1", [P, S * 4], F32, isOutput=False)
    idx2in = nc.declare_dram_parameter("idx2", [P, S * 8], I16, isOutput=False)
    msk2in = nc.declare_dram_parameter("msk2", [P, S * 4], F32, isOutput=False)
    xdin = nc.declare_dram_parameter("xd", [P, T * 2], F32, isOutput=False)
    cin = nc.declare_dram_parameter("consts", [P, CW], F32, isOutput=False)
    d1in = nc.declare_dram_parameter("dum1", [1, 64], F32, isOutput=False)
    d2in = nc.declare_dram_parameter("dum2", [1, 64], F32, isOutput=False)
    oext = nc.declare_dram_parameter("out", [T * P, 4], F32, isOutput=True)

    z1tab = nc.dram_tensor("z1tab", [R1, 64], F32)
    t2tab = nc.dram_tensor("t2tab", [R1, 64], F32, addr_space="Shared")
    z2sh = nc.dram_tensor("z2sh", [T * P, 16], F32)

    J = N_pad // P          # nodes per partition in the table build
    ACT = mybir.ActivationFunctionType
    ALU = mybir.AluOpType

    with tile.TileContext(nc) as tc, ExitStack() as ctx:
        persist = ctx.enter_context(tc.tile_pool(name="persist", bufs=1))
        build = ctx.enter_context(tc.tile_pool(name="build", bufs=1))
        iop = ctx.enter_context(tc.tile_pool(name="iop", bufs=3))
        gp = ctx.enter_context(tc.tile_pool(name="gath", bufs=2))
        zp = ctx.enter_context(tc.tile_pool(name="zp", bufs=2))
        wk = ctx.enter_context(tc.tile_pool(name="work", bufs=2))
        sm = ctx.enter_context(tc.tile_pool(name="small", bufs=3))
        pp = ctx.enter_context(tc.tile_pool(name="psA", bufs=2, space="PSUM"))
        pq = ctx.enter_context(tc.tile_pool(name="psB", bufs=2, space="PSUM"))

        csb = persist.tile([P, CW], F32)
        nc.sync.dma_start(out=csb[:], in_=cin[:])
        h3eS = persist.tile([P, T * 16], F32)
        nc.vector.memset(h3eS[:], 0.0)
        x_sb = persist.tile([P, J * 2], F32)
        nc.sync.dma_start(out=x_sb[:], in_=xin[:].rearrange("(p j) c -> p (j c)", p=P))
        adstE = persist.tile([P, T * 8], F32)
        nc.sync.dma_start(out=z1tab[R1 - 1:R1, :], in_=d1in[:])
        nc.sync.dma_start(out=t2tab[R1 - 1:R1, :], in_=d2in[:])
        tc.strict_bb_all_engine_barrier()

        # ---- prologue: build Z1 table (node-major: node = p*J + j), chunked ----
        nch = max(1, (J + 97) // 98)
        jc = (J + nch - 1) // nch
        for c0 in range(0, J, jc):
            jn = min(jc, J - c0)
            z1_sb = build.tile([P, jc * 16], F32, tag="zb")
            tt = build.tile([P, jc * 16], F32, tag="tb")
            x0b = _v(x_sb[:], c0 * 2, [[2, jn], [0, 16]])
            x1b = _v(x_sb[:], c0 * 2 + 1, [[2, jn], [0, 16]])
            as0b = _v(csb[:], AS0X, [[0, jn], [1, 16]])
            as1b = _v(csb[:], AS1X, [[0, jn], [1, 16]])
            z3 = _v(z1_sb[:], 0, [[16, jn], [1, 16]])
            t3 = _v(tt[:], 0, [[16, jn], [1, 16]])
            nc.vector.tensor_tensor(out=z3, in0=x0b, in1=as0b, op=ALU.mult)
            nc.vector.tensor_tensor(out=t3, in0=x1b, in1=as1b, op=ALU.mult)
            nc.vector.tensor_tensor(out=z3, in0=z3, in1=t3, op=ALU.add)
            nc.sync.dma_start(
                out=_dv(z1tab, c0 * 16, [[J * 16, P], [1, jn * 16]]),
                in_=z1_sb[:, 0:jn * 16])

        # a_dst per (partition, tile, head) from this core's dst-shard x rows
        xd = persist.tile([P, T * 2], F32)
        nc.sync.dma_start(out=xd[:], in_=xdin[:])
        ttd = build.tile([P, T * 8], F32, tag="td")
        nc.vector.tensor_tensor(
            out=adstE[:].rearrange("p (t h) -> p t h", h=8),
            in0=_v(xd[:], 0, [[2, T], [0, 8]]), in1=_v(csb[:], AD0, [[0, T], [1, 8]]),
            op=ALU.mult)
        nc.vector.tensor_tensor(
            out=ttd[:].rearrange("p (t h) -> p t h", h=8),
            in0=_v(xd[:], 1, [[2, T], [0, 8]]), in1=_v(csb[:], AD1, [[0, T], [1, 8]]),
            op=ALU.mult)
        nc.vector.tensor_tensor(out=adstE[:], in0=adstE[:], in1=ttd[:], op=ALU.add)

        tc.strict_bb_all_engine_barrier()

        def lrelu_exp(dst_t, src_t, n):
            if use_prelu:
                tmp = wk.tile([P, n], F32, tag="lrtmp")
                nc.scalar.activation(out=tmp[:], in_=src_t, func=ACT.Prelu, alpha=NEG)
                nc.scalar.activation(out=dst_t, in_=tmp[:], func=ACT.Exp)
            else:
                tmp = wk.tile([P, n], F32, tag="lrtmp")
                nc.vector.tensor_scalar_mul(tmp[:], src_t, NEG)
                nc.vector.tensor_tensor(out=tmp[:], in0=src_t, in1=tmp[:], op=ALU.max)
                nc.scalar.activation(out=dst_t, in_=tmp[:], func=ACT.Exp)

        def gather_extract(bi, b, tab, idxin, mskin, nval):
            """Gather block b's slot-columns from tab and mask-extract nval
            f32 per edge -> zblk [P, W*nval] (sub-entry cols 0..nval)."""
            (g0, ng, W, coff) = b
            idxt = iop.tile([P, BLK_COLS * 8], I16, tag="idx")
            nc.sync.dma_start(out=idxt[:, 0:W * 8], in_=idxin[:, coff * 8:(coff + W) * 8])
            mskt = iop.tile([P, BLK_COLS * 4], F32, tag="msk")
            nc.sync.dma_start(out=mskt[:, 0:W * 4], in_=mskin[:, coff * 4:(coff + W) * 4])
            raw = gp.tile([P, BLK_COLS * 64], F32, tag="raw")
            nc.gpsimd.dma_gather(
                raw[:, 0:W * 64].rearrange("p (w e) -> p w e", e=64),
                tab[:],
                idxt[:, 0:W * 8],
                W * P, W * P, 64,
                queue_num=bi % 4, single_packet=False)
            zblk = zp.tile([P, BLK_COLS * 10], F32, tag="z")
            tq = wk.tile([P, 3 * TQW], F32, tag="tq")
            outs = [_v(zblk[:], 0, [[nval, W], [1, nval]])] + [
                _v(tq[:], o * TQW, [[nval, W], [1, nval]]) for o in range(3)]
            for o in range(4):
                nc.vector.tensor_tensor(
                    out=outs[o],
                    in0=_v(raw[:], o * 16, [[64, W], [1, nval]]),
                    in1=_v(mskt[:], o, [[4, W], [0, nval]]),
                    op=ALU.mult)
            nc.vector.tensor_tensor(out=outs[1], in0=outs[1], in1=outs[2], op=ALU.add)
            nc.vector.tensor_tensor(out=outs[1], in0=outs[1], in1=outs[3], op=ALU.add)
            nc.vector.tensor_tensor(out=outs[0], in0=outs[0], in1=outs[1], op=ALU.add)
            return zblk

        # ---- layer 1 ----
        for bi, b in enumerate(blocks):
            (g0, ng, W, coff) = b
            zblk = gather_extract(bi, b, z1tab, idx1in, msk1in, 10)
            GnT = sm.tile([16, BLK_TILES * P], F32, tag="GnT")
            for gg in range(ng):
                (i0, D, off) = groups[g0 + gg]
                zb = (off - coff) * 10          # base f32 offset in zblk
                n8 = 8 * D
                e = wk.tile([P, n8], F32, tag="e1")
                nc.vector.tensor_tensor(
                    out=_v(e[:], 0, [[D, 8], [1, D]]),
                    in0=_v(zblk[:], zb, [[1, 8], [10, D]]),
                    in1=_v(adstE[:], i0 * 8, [[1, 8], [0, D]]),
                    op=ALU.add)
                ex = wk.tile([P, n8], F32, tag="ex1")
                lrelu_exp(ex[:], e[:], n8)
                s = sm.tile([P, 8], F32, tag="s1")
                nc.vector.tensor_reduce(
                    out=s[:], in_=ex[:].rearrange("p (a j) -> p a j", j=D),
                    axis=mybir.AxisListType.X, op=ALU.add)
                rs = sm.tile([P, 8], F32, tag="rs1")
                nc.vector.tensor_scalar_add(rs[:], s[:], EPS)
                nc.vector.reciprocal(rs[:], rs[:])
                prod = wk.tile([P, 2 * n8], F32, tag="pr1")
                nc.vector.tensor_tensor(
                    out=_v(prod[:], 0, [[n8, 2], [D, 8], [1, D]]),
                    in0=_v(ex[:], 0, [[0, 2], [D, 8], [1, D]]),
                    in1=_v(zblk[:], zb + 8, [[1, 2], [0, 8], [10, D]]),
                    op=ALU.mult)
                G = sm.tile([P, 16], F32, tag="G1")
                nc.vector.tensor_reduce(
                    out=G[:], in_=prod[:].rearrange("p (a j) -> p a j", j=D),
                    axis=mybir.AxisListType.X, op=ALU.add)
                Gn = sm.tile([P, 16], F32, tag="Gn1")
                nc.vector.tensor_tensor(
                    out=Gn[:].rearrange("p (k h) -> p k h", k=2),
                    in0=G[:].rearrange("p (k h) -> p k h", k=2),
                    in1=_v(rs[:], 0, [[0, 2], [1, 8]]),
                    op=ALU.mult)
                # transpose [d, (k,h)] -> [16, d] into the block's GnT strip
                pt = pp.tile([P, P], F32, tag="pt")
                nc.tensor.transpose(
                    out=pt[0:16, :],
                    in_=_v(Gn[:], 0, [[8, 2], [1, 8]]),
                    identity=csb[:, IDENT:IDENT + 128])
                nc.scalar.copy(out=GnT[0:16, gg * P:(gg + 1) * P], in_=pt[0:16, :])
            # expand through W1 (one matmul per block), then W2ext per tile
            o1p = pq.tile([P, BLK_TILES * P], F32, tag="o1p")
            nc.tensor.matmul(
                out=o1p[:, 0:ng * P],
                lhsT=csb[0:16, W1BLK:W1BLK + 128],
                rhs=GnT[0:16, 0:ng * P],
                start=True, stop=True)
            h2T = wk.tile([P, BLK_TILES * P], F32, tag="h2T")
            nc.scalar.activation(
                out=h2T[:, 0:ng * P], in_=o1p[:, 0:ng * P],
                func=ACT.Relu, bias=csb[:, B1:B1 + 1], scale=1.0)
            h3p = pq.tile([P, BLK_TILES * 8], F32, tag="h3p")
            for gg in range(ng):
                (i0, D, off) = groups[g0 + gg]
                nc.tensor.matmul(
                    out=h3p[:, gg * 8:gg * 8 + 6],
                    lhsT=h2T[:, gg * P:(gg + 1) * P],
                    rhs=csb[:, W2EXT:W2EXT + 6],
                    start=True, stop=True)
            nc.vector.tensor_copy(
                out=_v(h3eS[:], g0 * 16, [[16, ng], [1, 6]]),
                in_=_v(h3p[:], 0, [[8, ng], [1, 6]]))

        # ---- share T2 ----
        nc.sync.dma_start(
            out=_dv(z2sh, 0, [[16, P], [P * 16, T], [1, 16]]),
            in_=h3eS[:].rearrange("p (t c) -> p t c", c=16))
        tc.strict_bb_all_engine_barrier()
        nc.gpsimd.collective_compute(
            "AllGather", ALU.bypass,
            replica_groups=[list(range(NCORE))],
            ins=[z2sh[:]], outs=[t2tab[0:R1 - 1, :]])
        tc.strict_bb_all_engine_barrier()

        # ---- layer 2 ----
        for bi, b in enumerate(blocks):
            (g0, ng, W, coff) = b
            z2blk = gather_extract(bi, b, t2tab, idx2in, msk2in, 5)
            for gg in range(ng):
                (i0, D, off) = groups[g0 + gg]
                zb = (off - coff) * 5
                e2 = wk.tile([P, D], F32, tag="e2")
                nc.vector.tensor_tensor(
                    out=_v(e2[:], 0, [[1, D]]),
                    in0=_v(z2blk[:], zb + 4, [[5, D]]),
                    in1=_v(h3eS[:], i0 * 16 + 5, [[0, D]]),
                    op=ALU.add)
                ex2 = wk.tile([P, D], F32, tag="ex2")
                lrelu_exp(ex2[:], e2[:], D)
                s2 = sm.tile([P, 1], F32, tag="s2")
                nc.vector.tensor_reduce(
                    out=s2[:], in_=ex2[:],
                    axis=mybir.AxisListType.X, op=ALU.add)
                rs2 = sm.tile([P, 1], F32, tag="rs2")
                nc.vector.tensor_scalar_add(rs2[:], s2[:], EPS)
                nc.vector.reciprocal(rs2[:], rs2[:])
                prod2 = wk.tile([P, 4 * D], F32, tag="pr2")
                nc.vector.tensor_tensor(
                    out=_v(prod2[:], 0, [[D, 4], [1, D]]),
                    in0=_v(ex2[:], 0, [[0, 4], [1, D]]),
                    in1=_v(z2blk[:], zb, [[1, 4], [5, D]]),
                    op=ALU.mult)
                M2 = sm.tile([P, 4], F32, tag="M2")
                nc.vector.tensor_reduce(
                    out=M2[:], in_=prod2[:].rearrange("p (a j) -> p a j", j=D),
                    axis=mybir.AxisListType.X, op=ALU.add)
                o2 = sm.tile([P, 4], F32, tag="o2")
                nc.vector.tensor_tensor(
                    out=o2[:],
                    in0=M2[:],
                    in1=_v(rs2[:], 0, [[0, 4]]),
                    op=ALU.mult)
                nc.vector.tensor_tensor(
                    out=o2[:], in0=o2[:],
                    in1=_v(csb[:], B2, [[1, 4]]),
                    op=ALU.add)
                # log_softmax over c
                mx = sm.tile([P, 1], F32, tag="mx")
                nc.vector.tensor_reduce(
                    out=mx[:], in_=o2[:],
                    axis=mybir.AxisListType.X, op=ALU.max)
                z = sm.tile([P, 4], F32, tag="z")
                nc.vector.tensor_tensor(
                    out=z[:], in0=o2[:],
                    in1=_v(mx[:], 0, [[0, 4]]),
                    op=ALU.subtract)
                ez = sm.tile([P, 4], F32, tag="ez")
                nc.scalar.activation(out=ez[:], in_=z[:], func=ACT.Exp)
                se = sm.tile([P, 1], F32, tag="se")
                nc.vector.tensor_reduce(
                    out=se[:], in_=ez[:],
                    axis=mybir.AxisListType.X, op=ALU.add)
                lse = sm.tile([P, 1], F32, tag="lse")
                nc.scalar.activation(out=lse[:], in_=se[:], func=ACT.Ln)
                res = sm.tile([P, 4], F32, tag="res")
                nc.vector.tensor_tensor(
                    out=res[:], in0=z[:],
                    in1=_v(lse[:], 0, [[0, 4]]),
                    op=ALU.subtract)
                nc.sync.dma_start(
                    out=_dv(oext, i0 * P * 4, [[4, P], [1, 4]]),
                    in_=res[:])

    nc.compile()
    return nc


def kernel(**inputs) -> np.ndarray:
    x = np.asarray(inputs["x"], np.float32)
    edge_index = np.asarray(inputs["edge_index"])
    N = x.shape[0]
    src = edge_index[0].astype(np.int64)
    dst = edge_index[1].astype(np.int64)

    plan = _plan(src, dst, N)
    T, S, N_pad = plan["T"], plan["S"], plan["N_pad"]

    consts, dum1, dum2 = _consts(
        np.asarray(inputs["W1"], np.float32), np.asarray(inputs["att_src1"], np.float32),
        np.asarray(inputs["att_dst1"], np.float32), np.asarray(inputs["b1"], np.float32),
        np.asarray(inputs["W2"], np.float32), np.asarray(inputs["att_src2"], np.float32),
        np.asarray(inputs["att_dst2"], np.float32), np.asarray(inputs["b2"], np.float32))

    xpad = np.zeros((N_pad, 2), np.float32)
    xpad[:N] = x

    use_prelu = os.environ.get("GAT_NO_PRELU", "0") != "1"
    nc = _build(T, S, plan["groups"], plan["blocks"], N_pad, use_prelu=use_prelu)

    in_maps = []
    for c in range(NCORE):
        in_maps.append({
            "xpad": xpad,
            "idx1": plan["idx1"][c],
            "msk1": plan["msk1"][c],
            "idx2": plan["idx2"][c],
            "msk2": plan["msk2"][c],
            "xd": xpad[plan["dstid"][c]].reshape(P, -1),
            "consts": consts,
            "dum1": dum1,
            "dum2": dum2,
        })

    if os.environ.get("GAT_SIM", "0") == "1":
        from concourse.bass_interp import MultiCoreSim
        sim = MultiCoreSim(nc, NCORE)
        for c in range(NCORE):
            for k, v in in_maps[c].items():
                sim.cores[c].tensor(k)[:] = v
        sim.simulate()
        outs = [np.array(sim.cores[c].tensor("out")[:]) for c in range(NCORE)]
    else:
        trace = os.environ.get("GAT_TRACE", "0") == "1"
        res = run_bass_kernel_spmd(nc, in_maps, list(range(NCORE)), trace=trace)
        if trace:
            print(f"HW exec time: {res.exec_time_ns} ns")
        outs = [res.results[c]["out"] for c in range(NCORE)]

    big = np.concatenate(outs, axis=0)          # [NCORE*T*P, 4] core-major
    full = np.empty((N_pad, 4), np.float32)
    q = np.arange(N_pad)
    full[plan["order"][q]] = big[plan["pos_of_q"][q]]
    return full[:N]
